# revision 11
# baseline (speedup 1.0000x reference)
"""BiGRU (N=64, T=512, D=512, H=512) on 8 TRN2 NeuronCores.

Sharding: each core owns ONE direction (cores 0-3 fwd, 4-7 bwd) and a
16-sequence batch slice. Time is split into 4 chunks per direction with a
32-step cold-start warmup (GRU state decays to float noise in ~32 steps),
giving 64 lanes/core in ONE lockstep group and a scan of L = 152 steps.

The 64 lanes ride together as the M=64 stationary of every matmul, so the
weight matrices stream through the PE exactly once per step (the moving
operand streams 2 bf16 columns/cycle; measured col-tiling gives no real
stream concurrency, so minimizing streamed columns is what matters).

Layout per step: gates psum = 2 banks [128, 512] f32, bank b holding
strips 2b (partitions 0:64) and 2b+1 (partitions 64:128); regions
[z | r | hg | xg] per strip. h-side: 16 MMs N=384 (4 strips x 4 k-chunks,
M=64); x-side: 32 MMs (zr N=256 + xg N=128) issued one step ahead.
Elementwise per bank: sigmoid(r), sigmoid(z) -> t1=r*hg, gp=t1+xg ->
g=tanh(gp) -> d=h-g, m=z*d, hb=bf16(m+g), h'=m+g (f32 state). hb feeds
the PE transpose (4 col-tiled MMs vs I128 -> pt -> hT = next stationary)
and the output DMA. Bank B's chain trails bank A's by ~0.7us, hiding the
serial latency under bank A's matmul stream of the next step.
"""

from contextlib import ExitStack

import numpy as np
import ml_dtypes

import concourse.bacc as bacc
import concourse.bass as bass
import concourse.tile as tile
import concourse.mybir as mybir
from concourse import bass_utils

F32 = mybir.dt.float32
BF16 = mybir.dt.bfloat16
AF = mybir.ActivationFunctionType
ALU = mybir.AluOpType

N_CORES = 8
N, T, D, H = 64, 512, 512, 512
NCHUNK = 4            # time chunks per direction
WARM = 32             # cold-start warmup steps
L = (T + (NCHUNK - 1) * WARM) // NCHUNK  # 152 scan steps
STRIDE = L - WARM     # chunk start stride = 120
U = 4                 # steps per x-DMA block
BODY = 2 * U          # steps per For_i body (two blocks, A/B buffers)
NBLK = L // U         # 38 x blocks
NBODY = L // BODY     # 19 loop iterations


def build_bigru(repeats=1, with_bias=False, nbody=NBODY):
    assert L % BODY == 0
    nc = bacc.Bacc("TRN2", target_bir_lowering=False, debug=False,
                   num_devices=N_CORES)
    # x: [NBLK+2, 128, U*256]: step col = 64*k + lane (lane = 16*chunk+seq)
    xs = nc.dram_tensor("x", [NBLK + 2, 128, U * 256], BF16,
                        kind="ExternalInput").ap()
    outs = [nc.dram_tensor(f"out{b}", [L, 128, 128], BF16,
                           kind="ExternalOutput").ap() for b in range(2)]
    wx_d = nc.dram_tensor("wx", [4, 128, 1536], BF16, kind="ExternalInput").ap()
    wh_d = nc.dram_tensor("wh", [4, 128, 1536], BF16, kind="ExternalInput").ap()
    ident_d = nc.dram_tensor("ident", [128, 128], BF16,
                             kind="ExternalInput").ap()
    if with_bias:
        b_d = nc.dram_tensor("b", [1, 1536], BF16, kind="ExternalInput").ap()

    with tile.TileContext(nc) as tc, ExitStack() as ctx:
        cpool = ctx.enter_context(tc.tile_pool(name="const", bufs=1))
        pspools = [ctx.enter_context(
            tc.tile_pool(name=f"ps{b}", bufs=2, space="PSUM"))
            for b in range(2)]
        ptpools = [ctx.enter_context(
            tc.tile_pool(name=f"pt{b}", bufs=2, space="PSUM"))
            for b in range(2)]
        epool = ctx.enter_context(tc.tile_pool(name="elem", bufs=2))

        ident = cpool.tile([128, 128], BF16, tag="ident")
        nc.sync.dma_start(ident[:], ident_d[:])
        wx_sb = [cpool.tile([128, 1536], BF16, tag=f"wxk{k}", name=f"wxk{k}")
                 for k in range(4)]
        wh_sb = [cpool.tile([128, 1536], BF16, tag=f"whk{k}", name=f"whk{k}")
                 for k in range(4)]
        for k in range(4):
            nc.sync.dma_start(wx_sb[k][:], wx_d[k])
            nc.sync.dma_start(wh_sb[k][:], wh_d[k])
        if with_bias:
            b_sb = cpool.tile([1, 1536], BF16, tag="b")
            nc.sync.dma_start(b_sb[:], b_d[:])
            ones = cpool.tile([1, 64], BF16, tag="ones")
            nc.vector.memset(ones[:], 1.0)

        # persistent state, per bank b (bank b = strips 2b, 2b+1 = units
        # [256b, 256b+256); partitions = 64*(strip%2) + lane)
        h_state = [cpool.tile([128, 128], F32, tag=f"h{b}", name=f"h{b}")
                   for b in range(2)]
        hT = [cpool.tile([128, 128], BF16, tag=f"hT{b}", name=f"hT{b}")
              for b in range(2)]
        hb = [[cpool.tile([128, 128], BF16, tag=f"hb{b}{p}", name=f"hb{b}{p}")
               for p in range(2)] for b in range(2)]
        xbuf = [cpool.tile([128, U * 256], BF16, tag=f"xb{p}", name=f"xb{p}")
                for p in range(2)]
        for b in range(2):
            nc.vector.memset(h_state[b][:], 0.0)
            nc.vector.memset(hT[b][:], 0.0)
            nc.vector.memset(hb[b][0][:], 0.0)
            nc.vector.memset(hb[b][1][:], 0.0)

        def emit_x_mms(ps2, xcol, strips):
            """x-side matmuls into (psA, psB) for the given strips;
            opens each strip's accumulation group."""
            buf, col = xcol
            sx = xbuf[buf]
            for j in strips:
                ps = ps2[j // 2]
                po = 64 * (j % 2)
                mms = []
                for k in range(4):
                    lhs = sx[:, col * 256 + 64 * k:col * 256 + 64 * k + 64]
                    mms.append((ps[po:po + 64, 0:256], lhs,
                                wx_sb[k][:, 384 * j:384 * j + 256]))
                    mms.append((ps[po:po + 64, 384:512], lhs,
                                wx_sb[k][:, 384 * j + 256:384 * j + 384]))
                if with_bias:
                    mms.append((ps[po:po + 64, 0:256], ones[:],
                                b_sb[:, 384 * j:384 * j + 256]))
                    mms.append((ps[po:po + 64, 384:512], ones[:],
                                b_sb[:, 384 * j + 256:384 * j + 384]))
                for idx, (o, lt, rh) in enumerate(mms):
                    nc.tensor.matmul(o, lhsT=lt, rhs=rh,
                                     start=(idx == 0), stop=False,
                                     tile_position=(0, po))

        def emit_h_mms(ps2, strips):
            """h-side matmuls (z|r|hg, N=384, M=64) closing the groups."""
            for j in strips:
                ps = ps2[j // 2]
                po = 64 * (j % 2)
                for k in range(4):
                    nc.tensor.matmul(
                        ps[po:po + 64, 0:384],
                        lhsT=hT[k // 2][:, 64 * (k % 2):64 * (k % 2) + 64],
                        rhs=wh_sb[k][:, 384 * j:384 * j + 384],
                        start=False, stop=(k == 3),
                        tile_position=(0, po))

        def emit_tr(b, par):
            """Transpose hb[b][par] -> pt -> hT[b] (ACT copy, bf16)."""
            pt = ptpools[b].tile([128, 128], F32, tag=f"pt{b}", name="pt")
            for mb in range(4):
                nc.tensor.matmul(
                    pt[32 * mb:32 * mb + 32, :],
                    lhsT=hb[b][par][:, 32 * mb:32 * mb + 32],
                    rhs=ident[:], start=True, stop=True,
                    tile_position=(0, 32 * mb))
            nc.scalar.copy(hT[b][:], pt[:])

        def emit_sig(b, ps, which):
            t = epool.tile([128, 128], BF16, tag=f"{which}{b}", name=which)
            off = 128 if which == "r" else 0
            nc.scalar.activation(t[:], ps[:, off:off + 128], AF.Sigmoid)
            return t

        def emit_t1gp(b, ps, r):
            t1 = epool.tile([128, 128], BF16, tag=f"t1{b}", name="t1")
            nc.vector.tensor_tensor(t1[:], r[:], ps[:, 256:384], ALU.mult)
            gp = epool.tile([128, 128], BF16, tag=f"gp{b}", name="gp")
            nc.vector.tensor_tensor(gp[:], t1[:], ps[:, 384:512], ALU.add)
            return gp

        def emit_tanh(b, gp):
            g = epool.tile([128, 128], BF16, tag=f"g{b}", name="g")
            nc.scalar.activation(g[:], gp[:], AF.Tanh)
            return g

        def emit_blend(b, z, g, par):
            d = epool.tile([128, 128], F32, tag=f"d{b}", name="d")
            nc.vector.tensor_tensor(d[:], h_state[b][:], g[:], ALU.subtract)
            m = epool.tile([128, 128], F32, tag=f"m{b}", name="m")
            nc.vector.tensor_tensor(m[:], z[:], d[:], ALU.mult)
            nc.vector.tensor_tensor(hb[b][par][:], m[:], g[:], ALU.add)
            return m

        def emit_hupd(b, m, g):
            nc.vector.tensor_tensor(h_state[b][:], m[:], g[:], ALU.add)

        def emit_out(b, par, t_dyn):
            nc.sync.dma_start(outs[b][bass.ds(t_dyn, 1)].rearrange(
                "o p f -> (o p) f"), hb[b][par][:])

        # --- prologue: stage x block 0 ---
        nc.sync.dma_start(xbuf[0][:], xs[0])
        ps_pending = [None]

        def body(i):
            nc.sync.dma_start(
                xbuf[1][:],
                xs[bass.ds(2 * i + 1, 1)].rearrange("o p f -> (o p) f"))
            for tl in range(BODY):
                if tl == U:
                    nc.sync.dma_start(
                        xbuf[0][:],
                        xs[bass.ds(2 * i + 2, 1)].rearrange(
                            "o p f -> (o p) f"))
                t_dyn = i * BODY + tl
                nxt = tl + 1
                xcol = (1 if nxt % BODY >= U else 0, nxt % U)
                par, prev = tl % 2, (tl + 1) % 2
                # Emission follows predicted execution order (strict-FIFO
                # engine queues). Bank B's chain trails bank A's.
                emit_tr(0, prev)
                if tl == 0:
                    ps2 = (pspools[0].tile([128, 512], F32, tag="psA",
                                           name="ps"),
                           pspools[1].tile([128, 512], F32, tag="psB",
                                           name="ps"))
                    emit_x_mms(ps2, (0, 0), range(4))
                    ps_pending[0] = ps2
                emit_tr(1, prev)
                if tl < BODY - 1:
                    ps2n = (pspools[0].tile([128, 512], F32, tag="psA",
                                            name="ps"),
                            pspools[1].tile([128, 512], F32, tag="psB",
                                            name="ps"))
                    emit_x_mms(ps2n, xcol, range(4))
                psA, psB = ps_pending[0]
                emit_h_mms(ps_pending[0], (0, 1))
                rA = emit_sig(0, psA, "r")
                zA = emit_sig(0, psA, "z")
                emit_h_mms(ps_pending[0], (2, 3))
                gpA = emit_t1gp(0, psA, rA)
                gA = emit_tanh(0, gpA)
                rB = emit_sig(1, psB, "r")
                zB = emit_sig(1, psB, "z")
                mA = emit_blend(0, zA, gA, par)
                gpB = emit_t1gp(1, psB, rB)
                emit_hupd(0, mA, gA)
                gB = emit_tanh(1, gpB)
                mB = emit_blend(1, zB, gB, par)
                emit_hupd(1, mB, gB)
                emit_out(0, par, t_dyn)
                emit_out(1, par, t_dyn)
                if tl < BODY - 1:
                    ps_pending[0] = ps2n

        if repeats == 1:
            with tc.For_i(0, nbody) as i:
                body(i)
        else:
            with tc.For_i(0, repeats) as rr:
                with tc.For_i(0, nbody) as i:
                    body(i)
    nc.compile()
    return nc


def arrange_w(w):
    """[512, 1536] -> [4, 128, 1536]: k-chunk, d', strip-major [z|r|g]."""
    w = np.asarray(w, np.float32).reshape(4, 128, 3, 4, 128)
    w = w.transpose(0, 1, 3, 2, 4).reshape(4, 128, 1536)
    return np.ascontiguousarray(w).astype(ml_dtypes.bfloat16)


def arrange_b(b):
    b = np.asarray(b, np.float32).reshape(3, 4, 128).transpose(1, 0, 2)
    return np.ascontiguousarray(b.reshape(1, 1536)).astype(ml_dtypes.bfloat16)


def arrange_x_core(xd, seq0):
    """Per-core x tensor [NBLK+2, 128, U*256] bf16.

    xd: [N, T, D] (time-flipped for bwd cores). Step col 64*k + 16*j + s
    holds xd[seq0+s, STRIDE*j + i, 128*k + dd] on partition dd.
    """
    parts = []
    for j in range(NCHUNK):
        seg = xd[seq0:seq0 + 16, STRIDE * j:STRIDE * j + L, :]
        # [16, L, 512] -> [L, dd(128), k(4), s(16)]
        parts.append(seg.reshape(16, L, 4, 128).transpose(1, 3, 2, 0))
    # [L, 128, k(4), j(4), s(16)] -> [L, 128, 256]
    arr = np.stack(parts, axis=3).reshape(L, 128, 256)
    arr = arr.reshape(NBLK, U, 128, 256).transpose(0, 2, 1, 3).reshape(
        NBLK, 128, U * 256)
    full = np.zeros((NBLK + 2, 128, U * 256), np.float32)
    full[:NBLK] = arr
    return np.ascontiguousarray(full).astype(ml_dtypes.bfloat16)


def decode_out_core(oA, oB):
    """Two [L, 128, 128] bf16 outputs -> [16, T, H] f32 for this core.

    o[b][t, 64*sp + 16*j + s, d] = h(chunk j, seq s, t, unit 128*(2b+sp)+d)
    """
    a = np.stack([np.asarray(o, ml_dtypes.bfloat16).astype(np.float32)
                  for o in (oA, oB)], axis=1)
    a = a.reshape(L, 2, 2, 4, 16, 128).transpose(3, 4, 0, 1, 2, 5).reshape(
        NCHUNK, 16, L, 512)
    h = np.empty((16, T, H), np.float32)
    for j in range(NCHUNK):
        lo = 0 if j == 0 else WARM
        h[:, STRIDE * j + lo:STRIDE * j + L, :] = a[j][:, lo:, :]
    return h


def make_ident():
    return np.eye(128, dtype=ml_dtypes.bfloat16)


_CACHE = {}


def _get_program(with_bias):
    key = ("prog", with_bias)
    if key not in _CACHE:
        _CACHE[key] = build_bigru(repeats=1, with_bias=with_bias)
    return _CACHE[key]


def kernel(x, W_x_fwd, W_h_fwd, b_fwd, W_x_bwd, W_h_bwd, b_bwd):
    x = np.asarray(x, np.float32)
    assert x.shape == (N, T, D), x.shape
    b_fwd = np.asarray(b_fwd, np.float32)
    b_bwd = np.asarray(b_bwd, np.float32)
    with_bias = bool(np.any(b_fwd) or np.any(b_bwd))
    nc = _get_program(with_bias)

    x_rev = x[:, ::-1]
    wmaps = [
        {"wx": arrange_w(W_x_fwd), "wh": arrange_w(W_h_fwd)},
        {"wx": arrange_w(W_x_bwd), "wh": arrange_w(W_h_bwd)},
    ]
    if with_bias:
        wmaps[0]["b"] = arrange_b(b_fwd)
        wmaps[1]["b"] = arrange_b(b_bwd)
    ident = make_ident()
    in_maps = []
    for c in range(N_CORES):
        d = c // 4
        seq0 = 16 * (c % 4)
        m = dict(wmaps[d])
        m["ident"] = ident
        m["x"] = arrange_x_core(x_rev if d else x, seq0)
        in_maps.append(m)

    res = bass_utils.run_bass_kernel_spmd(nc, in_maps,
                                          core_ids=list(range(N_CORES)))
    out = np.empty((N, T, 2 * H), np.float32)
    for c in range(N_CORES):
        d = c // 4
        seq0 = 16 * (c % 4)
        h = decode_out_core(res.results[c]["out0"], res.results[c]["out1"])
        if d == 0:
            out[seq0:seq0 + 16, :, :H] = h
        else:
            out[seq0:seq0 + 16, :, H:] = h[:, ::-1]
    return out


# revision 13
# speedup vs baseline: 1.0805x; 1.0805x over previous
"""BiGRU (N=64, T=512, D=512, H=512) on 8 TRN2 NeuronCores.

Sharding: each core owns ONE direction (cores 0-3 fwd, 4-7 bwd) and a
16-sequence batch slice. Time is split into 4 chunks per direction with a
32-step cold-start warmup (GRU state decays to float noise in ~32 steps),
giving 64 lanes/core in ONE lockstep group and a scan of L = 152 steps.

The 64 lanes ride together as the M=64 stationary of every matmul, so the
weight matrices stream through the PE exactly once per step (the moving
operand streams 2 bf16 columns/cycle; measured col-tiling gives no real
stream concurrency, so minimizing streamed columns is what matters).

Layout per step: gates psum = 2 banks [128, 512] f32, bank b holding
strips 2b (partitions 0:64) and 2b+1 (partitions 64:128); regions
[z | r | hg | xg] per strip. h-side: 16 MMs N=384 (4 strips x 4 k-chunks,
M=64); x-side: 32 MMs (zr N=256 + xg N=128) issued one step ahead.
Elementwise per bank: sigmoid(r), sigmoid(z) -> t1=r*hg, gp=t1+xg ->
g=tanh(gp) -> d=h-g, m=z*d, hb=bf16(m+g), h'=m+g (f32 state). hb feeds
the PE transpose (4 col-tiled MMs vs I128 -> pt -> hT = next stationary)
and the output DMA. Bank B's chain trails bank A's by ~0.7us, hiding the
serial latency under bank A's matmul stream of the next step.
"""

from contextlib import ExitStack

import numpy as np
import ml_dtypes

import concourse.bacc as bacc
import concourse.bass as bass
import concourse.tile as tile
import concourse.mybir as mybir
from concourse import bass_utils

F32 = mybir.dt.float32
BF16 = mybir.dt.bfloat16
AF = mybir.ActivationFunctionType
ALU = mybir.AluOpType

N_CORES = 8
N, T, D, H = 64, 512, 512, 512
NCHUNK = 4            # time chunks per direction
WARM = 32             # cold-start warmup steps
L = (T + (NCHUNK - 1) * WARM) // NCHUNK  # 152 scan steps
STRIDE = L - WARM     # chunk start stride = 120
U = 4                 # steps per x-DMA block
BODY = 2 * U          # steps per For_i body (two blocks, A/B buffers)
NBLK = L // U         # 38 x blocks
NBODY = L // BODY     # 19 loop iterations


def build_bigru(repeats=1, with_bias=False, nbody=NBODY):
    assert L % BODY == 0
    nc = bacc.Bacc("TRN2", target_bir_lowering=False, debug=False,
                   num_devices=N_CORES)
    # x: [NBLK+2, 128, U*256]: step col = 64*k + lane (lane = 16*chunk+seq)
    xs = nc.dram_tensor("x", [NBLK + 2, 128, U * 256], BF16,
                        kind="ExternalInput").ap()
    outs = [nc.dram_tensor(f"out{b}", [L, 128, 128], BF16,
                           kind="ExternalOutput").ap() for b in range(2)]
    wx_d = nc.dram_tensor("wx", [4, 128, 1536], BF16, kind="ExternalInput").ap()
    wh_d = nc.dram_tensor("wh", [4, 128, 1536], BF16, kind="ExternalInput").ap()
    ident_d = nc.dram_tensor("ident", [128, 128], BF16,
                             kind="ExternalInput").ap()
    if with_bias:
        b_d = nc.dram_tensor("b", [1, 1536], BF16, kind="ExternalInput").ap()

    with tile.TileContext(nc) as tc, ExitStack() as ctx:
        cpool = ctx.enter_context(tc.tile_pool(name="const", bufs=1))
        pspools = [ctx.enter_context(
            tc.tile_pool(name=f"ps{b}", bufs=2, space="PSUM"))
            for b in range(2)]
        ptpools = [ctx.enter_context(
            tc.tile_pool(name=f"pt{b}", bufs=2, space="PSUM"))
            for b in range(2)]
        epool = ctx.enter_context(tc.tile_pool(name="elem", bufs=2))

        ident = cpool.tile([128, 128], BF16, tag="ident")
        nc.sync.dma_start(ident[:], ident_d[:])
        wx_sb = [cpool.tile([128, 1536], BF16, tag=f"wxk{k}", name=f"wxk{k}")
                 for k in range(4)]
        wh_sb = [cpool.tile([128, 1536], BF16, tag=f"whk{k}", name=f"whk{k}")
                 for k in range(4)]
        for k in range(4):
            nc.sync.dma_start(wx_sb[k][:], wx_d[k])
            nc.sync.dma_start(wh_sb[k][:], wh_d[k])
        if with_bias:
            b_sb = cpool.tile([1, 1536], BF16, tag="b")
            nc.sync.dma_start(b_sb[:], b_d[:])
            ones = cpool.tile([1, 64], BF16, tag="ones")
            nc.vector.memset(ones[:], 1.0)

        # persistent state, per bank b (bank b = strips 2b, 2b+1 = units
        # [256b, 256b+256); partitions = 64*(strip%2) + lane)
        h_state = [cpool.tile([128, 128], F32, tag=f"h{b}", name=f"h{b}")
                   for b in range(2)]
        hT = [cpool.tile([128, 128], BF16, tag=f"hT{b}", name=f"hT{b}")
              for b in range(2)]
        hb = [[cpool.tile([128, 128], BF16, tag=f"hb{b}{p}", name=f"hb{b}{p}")
               for p in range(2)] for b in range(2)]
        xbuf = [cpool.tile([128, U * 256], BF16, tag=f"xb{p}", name=f"xb{p}")
                for p in range(2)]
        for b in range(2):
            nc.vector.memset(h_state[b][:], 0.0)
            nc.vector.memset(hT[b][:], 0.0)
            nc.vector.memset(hb[b][0][:], 0.0)
            nc.vector.memset(hb[b][1][:], 0.0)

        def emit_x_mms(ps2, xcol, strips):
            """x-side matmuls into (psA, psB) for the given strips;
            opens each strip's accumulation group."""
            buf, col = xcol
            sx = xbuf[buf]
            # k outer / strip inner: consecutive MMs alternate array column
            # halves (po 0/64) and reuse the same stationary per position,
            # so LDWEIGHTS overlaps the moving stream instead of
            # serializing with it.
            for k in range(4):
                lhs = sx[:, col * 256 + 64 * k:col * 256 + 64 * k + 64]
                for j in strips:
                    ps = ps2[j // 2]
                    po = 64 * (j % 2)
                    nc.tensor.matmul(ps[po:po + 64, 0:256], lhsT=lhs,
                                     rhs=wx_sb[k][:, 384 * j:384 * j + 256],
                                     start=(k == 0), stop=False,
                                     tile_position=(0, po))
                    nc.tensor.matmul(
                        ps[po:po + 64, 384:512], lhsT=lhs,
                        rhs=wx_sb[k][:, 384 * j + 256:384 * j + 384],
                        start=False, stop=False,
                        tile_position=(0, po))
            if with_bias:
                for j in strips:
                    ps = ps2[j // 2]
                    po = 64 * (j % 2)
                    nc.tensor.matmul(ps[po:po + 64, 0:256], lhsT=ones[:],
                                     rhs=b_sb[:, 384 * j:384 * j + 256],
                                     start=False, stop=False,
                                     tile_position=(0, po))
                    nc.tensor.matmul(
                        ps[po:po + 64, 384:512], lhsT=ones[:],
                        rhs=b_sb[:, 384 * j + 256:384 * j + 384],
                        start=False, stop=False,
                        tile_position=(0, po))

        def emit_h_mms(ps2, strips):
            """h-side matmuls (z|r|hg, N=384, M=64) closing the groups."""
            for k in range(4):
                lhs = hT[k // 2][:, 64 * (k % 2):64 * (k % 2) + 64]
                for j in strips:
                    ps = ps2[j // 2]
                    po = 64 * (j % 2)
                    nc.tensor.matmul(
                        ps[po:po + 64, 0:384], lhsT=lhs,
                        rhs=wh_sb[k][:, 384 * j:384 * j + 384],
                        start=False, stop=(k == 3),
                        tile_position=(0, po))

        def emit_tr(b, par):
            """Transpose hb[b][par] -> pt -> hT[b] (ACT copy, bf16)."""
            pt = ptpools[b].tile([128, 128], F32, tag=f"pt{b}", name="pt")
            for mb in range(4):
                nc.tensor.matmul(
                    pt[32 * mb:32 * mb + 32, :],
                    lhsT=hb[b][par][:, 32 * mb:32 * mb + 32],
                    rhs=ident[:], start=True, stop=True,
                    tile_position=(0, 32 * mb))
            nc.scalar.copy(hT[b][:], pt[:])

        def emit_sig(b, ps, which):
            t = epool.tile([128, 128], BF16, tag=f"{which}{b}", name=which)
            off = 128 if which == "r" else 0
            nc.scalar.activation(t[:], ps[:, off:off + 128], AF.Sigmoid)
            return t

        def emit_t1gp(b, ps, r):
            t1 = epool.tile([128, 128], BF16, tag=f"t1{b}", name="t1")
            nc.vector.tensor_tensor(t1[:], r[:], ps[:, 256:384], ALU.mult)
            gp = epool.tile([128, 128], BF16, tag=f"gp{b}", name="gp")
            nc.vector.tensor_tensor(gp[:], t1[:], ps[:, 384:512], ALU.add)
            return gp

        def emit_tanh(b, gp):
            g = epool.tile([128, 128], BF16, tag=f"g{b}", name="g")
            nc.scalar.activation(g[:], gp[:], AF.Tanh)
            return g

        def emit_blend(b, z, g, par):
            d = epool.tile([128, 128], F32, tag=f"d{b}", name="d")
            nc.vector.tensor_tensor(d[:], h_state[b][:], g[:], ALU.subtract)
            m = epool.tile([128, 128], F32, tag=f"m{b}", name="m")
            nc.vector.tensor_tensor(m[:], z[:], d[:], ALU.mult)
            nc.vector.tensor_tensor(hb[b][par][:], m[:], g[:], ALU.add)
            return m

        def emit_hupd(b, m, g):
            nc.vector.tensor_tensor(h_state[b][:], m[:], g[:], ALU.add)

        def emit_out(b, par, t_dyn):
            nc.sync.dma_start(outs[b][bass.ds(t_dyn, 1)].rearrange(
                "o p f -> (o p) f"), hb[b][par][:])

        # --- prologue: stage x block 0 ---
        nc.sync.dma_start(xbuf[0][:], xs[0])
        ps_pending = [None]

        def body(i):
            nc.sync.dma_start(
                xbuf[1][:],
                xs[bass.ds(2 * i + 1, 1)].rearrange("o p f -> (o p) f"))
            for tl in range(BODY):
                if tl == U:
                    nc.sync.dma_start(
                        xbuf[0][:],
                        xs[bass.ds(2 * i + 2, 1)].rearrange(
                            "o p f -> (o p) f"))
                t_dyn = i * BODY + tl
                nxt = tl + 1
                xcol = (1 if nxt % BODY >= U else 0, nxt % U)
                par, prev = tl % 2, (tl + 1) % 2
                # Emission follows predicted execution order (strict-FIFO
                # engine queues). Bank B's chain trails bank A's.
                emit_tr(0, prev)
                if tl == 0:
                    ps2 = (pspools[0].tile([128, 512], F32, tag="psA",
                                           name="ps"),
                           pspools[1].tile([128, 512], F32, tag="psB",
                                           name="ps"))
                    emit_x_mms(ps2, (0, 0), range(4))
                    ps_pending[0] = ps2
                emit_tr(1, prev)
                if tl < BODY - 1:
                    ps2n = (pspools[0].tile([128, 512], F32, tag="psA",
                                            name="ps"),
                            pspools[1].tile([128, 512], F32, tag="psB",
                                            name="ps"))
                    emit_x_mms(ps2n, xcol, range(4))
                psA, psB = ps_pending[0]
                emit_h_mms(ps_pending[0], (0, 1))
                rA = emit_sig(0, psA, "r")
                zA = emit_sig(0, psA, "z")
                emit_h_mms(ps_pending[0], (2, 3))
                gpA = emit_t1gp(0, psA, rA)
                gA = emit_tanh(0, gpA)
                rB = emit_sig(1, psB, "r")
                zB = emit_sig(1, psB, "z")
                mA = emit_blend(0, zA, gA, par)
                gpB = emit_t1gp(1, psB, rB)
                emit_hupd(0, mA, gA)
                gB = emit_tanh(1, gpB)
                mB = emit_blend(1, zB, gB, par)
                emit_hupd(1, mB, gB)
                emit_out(0, par, t_dyn)
                emit_out(1, par, t_dyn)
                if tl < BODY - 1:
                    ps_pending[0] = ps2n

        if repeats == 1:
            with tc.For_i(0, nbody) as i:
                body(i)
        else:
            with tc.For_i(0, repeats) as rr:
                with tc.For_i(0, nbody) as i:
                    body(i)
    nc.compile()
    return nc


def arrange_w(w):
    """[512, 1536] -> [4, 128, 1536]: k-chunk, d', strip-major [z|r|g]."""
    w = np.asarray(w, np.float32).reshape(4, 128, 3, 4, 128)
    w = w.transpose(0, 1, 3, 2, 4).reshape(4, 128, 1536)
    return np.ascontiguousarray(w).astype(ml_dtypes.bfloat16)


def arrange_b(b):
    b = np.asarray(b, np.float32).reshape(3, 4, 128).transpose(1, 0, 2)
    return np.ascontiguousarray(b.reshape(1, 1536)).astype(ml_dtypes.bfloat16)


def arrange_x_core(xd, seq0):
    """Per-core x tensor [NBLK+2, 128, U*256] bf16.

    xd: [N, T, D] (time-flipped for bwd cores). Step col 64*k + 16*j + s
    holds xd[seq0+s, STRIDE*j + i, 128*k + dd] on partition dd.
    """
    parts = []
    for j in range(NCHUNK):
        seg = xd[seq0:seq0 + 16, STRIDE * j:STRIDE * j + L, :]
        # [16, L, 512] -> [L, dd(128), k(4), s(16)]
        parts.append(seg.reshape(16, L, 4, 128).transpose(1, 3, 2, 0))
    # [L, 128, k(4), j(4), s(16)] -> [L, 128, 256]
    arr = np.stack(parts, axis=3).reshape(L, 128, 256)
    arr = arr.reshape(NBLK, U, 128, 256).transpose(0, 2, 1, 3).reshape(
        NBLK, 128, U * 256)
    full = np.zeros((NBLK + 2, 128, U * 256), np.float32)
    full[:NBLK] = arr
    return np.ascontiguousarray(full).astype(ml_dtypes.bfloat16)


def decode_out_core(oA, oB):
    """Two [L, 128, 128] bf16 outputs -> [16, T, H] f32 for this core.

    o[b][t, 64*sp + 16*j + s, d] = h(chunk j, seq s, t, unit 128*(2b+sp)+d)
    """
    a = np.stack([np.asarray(o, ml_dtypes.bfloat16).astype(np.float32)
                  for o in (oA, oB)], axis=1)
    a = a.reshape(L, 2, 2, 4, 16, 128).transpose(3, 4, 0, 1, 2, 5).reshape(
        NCHUNK, 16, L, 512)
    h = np.empty((16, T, H), np.float32)
    for j in range(NCHUNK):
        lo = 0 if j == 0 else WARM
        h[:, STRIDE * j + lo:STRIDE * j + L, :] = a[j][:, lo:, :]
    return h


def make_ident():
    return np.eye(128, dtype=ml_dtypes.bfloat16)


_CACHE = {}


def _get_program(with_bias):
    key = ("prog", with_bias)
    if key not in _CACHE:
        _CACHE[key] = build_bigru(repeats=1, with_bias=with_bias)
    return _CACHE[key]


def kernel(x, W_x_fwd, W_h_fwd, b_fwd, W_x_bwd, W_h_bwd, b_bwd):
    x = np.asarray(x, np.float32)
    assert x.shape == (N, T, D), x.shape
    b_fwd = np.asarray(b_fwd, np.float32)
    b_bwd = np.asarray(b_bwd, np.float32)
    with_bias = bool(np.any(b_fwd) or np.any(b_bwd))
    nc = _get_program(with_bias)

    x_rev = x[:, ::-1]
    wmaps = [
        {"wx": arrange_w(W_x_fwd), "wh": arrange_w(W_h_fwd)},
        {"wx": arrange_w(W_x_bwd), "wh": arrange_w(W_h_bwd)},
    ]
    if with_bias:
        wmaps[0]["b"] = arrange_b(b_fwd)
        wmaps[1]["b"] = arrange_b(b_bwd)
    ident = make_ident()
    in_maps = []
    for c in range(N_CORES):
        d = c // 4
        seq0 = 16 * (c % 4)
        m = dict(wmaps[d])
        m["ident"] = ident
        m["x"] = arrange_x_core(x_rev if d else x, seq0)
        in_maps.append(m)

    res = bass_utils.run_bass_kernel_spmd(nc, in_maps,
                                          core_ids=list(range(N_CORES)))
    out = np.empty((N, T, 2 * H), np.float32)
    for c in range(N_CORES):
        d = c // 4
        seq0 = 16 * (c % 4)
        h = decode_out_core(res.results[c]["out0"], res.results[c]["out1"])
        if d == 0:
            out[seq0:seq0 + 16, :, :H] = h
        else:
            out[seq0:seq0 + 16, :, H:] = h[:, ::-1]
    return out


# revision 14
# speedup vs baseline: 1.1060x; 1.0235x over previous
"""BiGRU (N=64, T=512, D=512, H=512) on 8 TRN2 NeuronCores.

Sharding: each core owns ONE direction (cores 0-3 fwd, 4-7 bwd) and a
16-sequence batch slice. Time is split into 4 chunks per direction with a
32-step cold-start warmup (GRU state decays to float noise in ~32 steps),
giving 64 lanes/core in ONE lockstep group and a scan of L = 152 steps.

The 64 lanes ride together as the M=64 stationary of every matmul, so the
weight matrices stream through the PE exactly once per step (the moving
operand streams 2 bf16 columns/cycle; measured col-tiling gives no real
stream concurrency, so minimizing streamed columns is what matters).

Layout per step: gates psum = 2 banks [128, 512] f32, bank b holding
strips 2b (partitions 0:64) and 2b+1 (partitions 64:128); regions
[z | r | hg | xg] per strip. h-side: 16 MMs N=384 (4 strips x 4 k-chunks,
M=64); x-side: 32 MMs (zr N=256 + xg N=128) issued one step ahead.
Elementwise per bank: sigmoid(r), sigmoid(z) -> t1=r*hg, gp=t1+xg ->
g=tanh(gp) -> d=h-g, m=z*d, hb=bf16(m+g), h'=m+g (f32 state). hb feeds
the PE transpose (4 col-tiled MMs vs I128 -> pt -> hT = next stationary)
and the output DMA. Bank B's chain trails bank A's by ~0.7us, hiding the
serial latency under bank A's matmul stream of the next step.
"""

from contextlib import ExitStack

import numpy as np
import ml_dtypes

import concourse.bacc as bacc
import concourse.bass as bass
import concourse.tile as tile
import concourse.mybir as mybir
from concourse import bass_utils

F32 = mybir.dt.float32
BF16 = mybir.dt.bfloat16
AF = mybir.ActivationFunctionType
ALU = mybir.AluOpType

N_CORES = 8
N, T, D, H = 64, 512, 512, 512
NCHUNK = 4            # time chunks per direction
WARM = 32             # cold-start warmup steps
L = (T + (NCHUNK - 1) * WARM) // NCHUNK  # 152 scan steps
STRIDE = L - WARM     # chunk start stride = 120
U = 4                 # steps per x-DMA block
BODY = 2 * U          # steps per For_i body (two blocks, A/B buffers)
NBLK = L // U         # 38 x blocks
NBODY = L // BODY     # 19 loop iterations


def build_bigru(repeats=1, with_bias=False, nbody=NBODY):
    assert L % BODY == 0
    nc = bacc.Bacc("TRN2", target_bir_lowering=False, debug=False,
                   num_devices=N_CORES)
    # x: [NBLK+2, 128, U*256]: step col = 64*k + lane (lane = 16*chunk+seq)
    xs = nc.dram_tensor("x", [NBLK + 2, 128, U * 256], BF16,
                        kind="ExternalInput").ap()
    outs = [nc.dram_tensor(f"out{b}", [L, 128, 128], BF16,
                           kind="ExternalOutput").ap() for b in range(2)]
    wx_d = nc.dram_tensor("wx", [4, 128, 1536], BF16, kind="ExternalInput").ap()
    wh_d = nc.dram_tensor("wh", [4, 128, 1536], BF16, kind="ExternalInput").ap()
    ident_d = nc.dram_tensor("ident", [128, 128], BF16,
                             kind="ExternalInput").ap()
    if with_bias:
        b_d = nc.dram_tensor("b", [1, 1536], BF16, kind="ExternalInput").ap()

    with tile.TileContext(nc) as tc, ExitStack() as ctx:
        cpool = ctx.enter_context(tc.tile_pool(name="const", bufs=1))
        pspools = [ctx.enter_context(
            tc.tile_pool(name=f"ps{b}", bufs=2, space="PSUM"))
            for b in range(2)]
        ptpools = [ctx.enter_context(
            tc.tile_pool(name=f"pt{b}", bufs=2, space="PSUM"))
            for b in range(2)]
        epool = ctx.enter_context(tc.tile_pool(name="elem", bufs=2))

        ident = cpool.tile([128, 128], BF16, tag="ident")
        nc.sync.dma_start(ident[:], ident_d[:])
        wx_sb = [cpool.tile([128, 1536], BF16, tag=f"wxk{k}", name=f"wxk{k}")
                 for k in range(4)]
        wh_sb = [cpool.tile([128, 1536], BF16, tag=f"whk{k}", name=f"whk{k}")
                 for k in range(4)]
        for k in range(4):
            nc.sync.dma_start(wx_sb[k][:], wx_d[k])
            nc.sync.dma_start(wh_sb[k][:], wh_d[k])
        if with_bias:
            b_sb = cpool.tile([1, 1536], BF16, tag="b")
            nc.sync.dma_start(b_sb[:], b_d[:])
            ones = cpool.tile([1, 64], BF16, tag="ones")
            nc.vector.memset(ones[:], 1.0)

        # persistent state, per bank b (bank b = strips 2b, 2b+1 = units
        # [256b, 256b+256); partitions = 64*(strip%2) + lane)
        h_state = [cpool.tile([128, 128], F32, tag=f"h{b}", name=f"h{b}")
                   for b in range(2)]
        hT = [cpool.tile([128, 128], BF16, tag=f"hT{b}", name=f"hT{b}")
              for b in range(2)]
        hb = [[cpool.tile([128, 128], BF16, tag=f"hb{b}{p}", name=f"hb{b}{p}")
               for p in range(2)] for b in range(2)]
        xbuf = [cpool.tile([128, U * 256], BF16, tag=f"xb{p}", name=f"xb{p}")
                for p in range(2)]
        for b in range(2):
            nc.vector.memset(h_state[b][:], 0.0)
            nc.vector.memset(hT[b][:], 0.0)
            nc.vector.memset(hb[b][0][:], 0.0)
            nc.vector.memset(hb[b][1][:], 0.0)

        def emit_x_mms(ps2, xcol, strips):
            """x-side matmuls into (psA, psB) for the given strips;
            opens each strip's accumulation group."""
            buf, col = xcol
            sx = xbuf[buf]
            # k outer / strip inner: consecutive MMs alternate array column
            # halves (po 0/64) and reuse the same stationary per position,
            # so LDWEIGHTS overlaps the moving stream instead of
            # serializing with it.
            for k in range(4):
                lhs = sx[:, col * 256 + 64 * k:col * 256 + 64 * k + 64]
                for j in strips:
                    ps = ps2[j // 2]
                    po = 64 * (j % 2)
                    nc.tensor.matmul(ps[po:po + 64, 0:256], lhsT=lhs,
                                     rhs=wx_sb[k][:, 384 * j:384 * j + 256],
                                     start=(k == 0), stop=False,
                                     tile_position=(0, po))
                    nc.tensor.matmul(
                        ps[po:po + 64, 384:512], lhsT=lhs,
                        rhs=wx_sb[k][:, 384 * j + 256:384 * j + 384],
                        start=False, stop=False,
                        tile_position=(0, po))
            if with_bias:
                for j in strips:
                    ps = ps2[j // 2]
                    po = 64 * (j % 2)
                    nc.tensor.matmul(ps[po:po + 64, 0:256], lhsT=ones[:],
                                     rhs=b_sb[:, 384 * j:384 * j + 256],
                                     start=False, stop=False,
                                     tile_position=(0, po))
                    nc.tensor.matmul(
                        ps[po:po + 64, 384:512], lhsT=ones[:],
                        rhs=b_sb[:, 384 * j + 256:384 * j + 384],
                        start=False, stop=False,
                        tile_position=(0, po))

        def emit_h_mms(ps2, strips):
            """h-side matmuls (z|r|hg, N=384, M=64) closing the groups."""
            for k in range(4):
                lhs = hT[k // 2][:, 64 * (k % 2):64 * (k % 2) + 64]
                for j in strips:
                    ps = ps2[j // 2]
                    po = 64 * (j % 2)
                    nc.tensor.matmul(
                        ps[po:po + 64, 0:384], lhsT=lhs,
                        rhs=wh_sb[k][:, 384 * j:384 * j + 384],
                        start=False, stop=(k == 3),
                        tile_position=(0, po))

        def emit_tr(b, par):
            """Transpose hb[b][par] -> pt -> hT[b] (ACT copy, bf16)."""
            pt = ptpools[b].tile([128, 128], F32, tag=f"pt{b}", name="pt")
            for mb in range(4):
                nc.tensor.matmul(
                    pt[32 * mb:32 * mb + 32, :],
                    lhsT=hb[b][par][:, 32 * mb:32 * mb + 32],
                    rhs=ident[:], start=True, stop=True,
                    tile_position=(0, 32 * mb))
            nc.scalar.copy(hT[b][:], pt[:])

        def emit_sig(b, ps, which):
            t = epool.tile([128, 128], BF16, tag=f"{which}{b}", name=which)
            off = 128 if which == "r" else 0
            nc.scalar.activation(t[:], ps[:, off:off + 128], AF.Sigmoid)
            return t

        def emit_t1gp(b, ps, r):
            t1 = epool.tile([128, 128], BF16, tag=f"t1{b}", name="t1")
            nc.vector.tensor_tensor(t1[:], r[:], ps[:, 256:384], ALU.mult)
            gp = epool.tile([128, 128], BF16, tag=f"gp{b}", name="gp")
            nc.vector.tensor_tensor(gp[:], t1[:], ps[:, 384:512], ALU.add)
            return gp

        def emit_tanh(b, gp):
            g = epool.tile([128, 128], BF16, tag=f"g{b}", name="g")
            nc.scalar.activation(g[:], gp[:], AF.Tanh)
            return g

        def emit_blend(b, z, g, par):
            d = epool.tile([128, 128], F32, tag=f"d{b}", name="d")
            nc.vector.tensor_tensor(d[:], h_state[b][:], g[:], ALU.subtract)
            m = epool.tile([128, 128], F32, tag=f"m{b}", name="m")
            nc.vector.tensor_tensor(m[:], z[:], d[:], ALU.mult)
            nc.vector.tensor_tensor(hb[b][par][:], m[:], g[:], ALU.add)
            return m

        def emit_hupd(b, m, g):
            nc.vector.tensor_tensor(h_state[b][:], m[:], g[:], ALU.add)

        def emit_out(b, par, t_dyn):
            nc.sync.dma_start(outs[b][bass.ds(t_dyn, 1)].rearrange(
                "o p f -> (o p) f"), hb[b][par][:])

        # --- prologue: stage x block 0 ---
        nc.sync.dma_start(xbuf[0][:], xs[0])
        ps_pending = [None]

        def body(i):
            nc.sync.dma_start(
                xbuf[1][:],
                xs[bass.ds(2 * i + 1, 1)].rearrange("o p f -> (o p) f"))
            for tl in range(BODY):
                if tl == U:
                    nc.sync.dma_start(
                        xbuf[0][:],
                        xs[bass.ds(2 * i + 2, 1)].rearrange(
                            "o p f -> (o p) f"))
                t_dyn = i * BODY + tl
                nxt = tl + 1
                xcol = (1 if nxt % BODY >= U else 0, nxt % U)
                par, prev = tl % 2, (tl + 1) % 2
                # Emission follows predicted execution order (strict-FIFO
                # engine queues). Bank B's chain trails bank A's.
                emit_tr(0, prev)
                if tl == 0:
                    ps2 = (pspools[0].tile([128, 512], F32, tag="psA",
                                           name="ps"),
                           pspools[1].tile([128, 512], F32, tag="psB",
                                           name="ps"))
                    emit_x_mms(ps2, (0, 0), range(4))
                    ps_pending[0] = ps2
                emit_tr(1, prev)
                psA, psB = ps_pending[0]
                emit_h_mms(ps_pending[0], (0, 1))
                rA = emit_sig(0, psA, "r")
                zA = emit_sig(0, psA, "z")
                emit_h_mms(ps_pending[0], (2, 3))
                # x-side of step t+1 goes to the PE AFTER step t's h-side,
                # so it streams during the elementwise chain instead of
                # delaying h(t) in the FIFO.
                if tl < BODY - 1:
                    ps2n = (pspools[0].tile([128, 512], F32, tag="psA",
                                            name="ps"),
                            pspools[1].tile([128, 512], F32, tag="psB",
                                            name="ps"))
                    emit_x_mms(ps2n, xcol, range(4))
                gpA = emit_t1gp(0, psA, rA)
                gA = emit_tanh(0, gpA)
                rB = emit_sig(1, psB, "r")
                zB = emit_sig(1, psB, "z")
                mA = emit_blend(0, zA, gA, par)
                gpB = emit_t1gp(1, psB, rB)
                emit_hupd(0, mA, gA)
                gB = emit_tanh(1, gpB)
                mB = emit_blend(1, zB, gB, par)
                emit_hupd(1, mB, gB)
                emit_out(0, par, t_dyn)
                emit_out(1, par, t_dyn)
                if tl < BODY - 1:
                    ps_pending[0] = ps2n

        if repeats == 1:
            with tc.For_i(0, nbody) as i:
                body(i)
        else:
            with tc.For_i(0, repeats) as rr:
                with tc.For_i(0, nbody) as i:
                    body(i)
    nc.compile()
    return nc


def arrange_w(w):
    """[512, 1536] -> [4, 128, 1536]: k-chunk, d', strip-major [z|r|g]."""
    w = np.asarray(w, np.float32).reshape(4, 128, 3, 4, 128)
    w = w.transpose(0, 1, 3, 2, 4).reshape(4, 128, 1536)
    return np.ascontiguousarray(w).astype(ml_dtypes.bfloat16)


def arrange_b(b):
    b = np.asarray(b, np.float32).reshape(3, 4, 128).transpose(1, 0, 2)
    return np.ascontiguousarray(b.reshape(1, 1536)).astype(ml_dtypes.bfloat16)


def arrange_x_core(xd, seq0):
    """Per-core x tensor [NBLK+2, 128, U*256] bf16.

    xd: [N, T, D] (time-flipped for bwd cores). Step col 64*k + 16*j + s
    holds xd[seq0+s, STRIDE*j + i, 128*k + dd] on partition dd.
    """
    parts = []
    for j in range(NCHUNK):
        seg = xd[seq0:seq0 + 16, STRIDE * j:STRIDE * j + L, :]
        # [16, L, 512] -> [L, dd(128), k(4), s(16)]
        parts.append(seg.reshape(16, L, 4, 128).transpose(1, 3, 2, 0))
    # [L, 128, k(4), j(4), s(16)] -> [L, 128, 256]
    arr = np.stack(parts, axis=3).reshape(L, 128, 256)
    arr = arr.reshape(NBLK, U, 128, 256).transpose(0, 2, 1, 3).reshape(
        NBLK, 128, U * 256)
    full = np.zeros((NBLK + 2, 128, U * 256), np.float32)
    full[:NBLK] = arr
    return np.ascontiguousarray(full).astype(ml_dtypes.bfloat16)


def decode_out_core(oA, oB):
    """Two [L, 128, 128] bf16 outputs -> [16, T, H] f32 for this core.

    o[b][t, 64*sp + 16*j + s, d] = h(chunk j, seq s, t, unit 128*(2b+sp)+d)
    """
    a = np.stack([np.asarray(o, ml_dtypes.bfloat16).astype(np.float32)
                  for o in (oA, oB)], axis=1)
    a = a.reshape(L, 2, 2, 4, 16, 128).transpose(3, 4, 0, 1, 2, 5).reshape(
        NCHUNK, 16, L, 512)
    h = np.empty((16, T, H), np.float32)
    for j in range(NCHUNK):
        lo = 0 if j == 0 else WARM
        h[:, STRIDE * j + lo:STRIDE * j + L, :] = a[j][:, lo:, :]
    return h


def make_ident():
    return np.eye(128, dtype=ml_dtypes.bfloat16)


_CACHE = {}


def _get_program(with_bias):
    key = ("prog", with_bias)
    if key not in _CACHE:
        _CACHE[key] = build_bigru(repeats=1, with_bias=with_bias)
    return _CACHE[key]


def kernel(x, W_x_fwd, W_h_fwd, b_fwd, W_x_bwd, W_h_bwd, b_bwd):
    x = np.asarray(x, np.float32)
    assert x.shape == (N, T, D), x.shape
    b_fwd = np.asarray(b_fwd, np.float32)
    b_bwd = np.asarray(b_bwd, np.float32)
    with_bias = bool(np.any(b_fwd) or np.any(b_bwd))
    nc = _get_program(with_bias)

    x_rev = x[:, ::-1]
    wmaps = [
        {"wx": arrange_w(W_x_fwd), "wh": arrange_w(W_h_fwd)},
        {"wx": arrange_w(W_x_bwd), "wh": arrange_w(W_h_bwd)},
    ]
    if with_bias:
        wmaps[0]["b"] = arrange_b(b_fwd)
        wmaps[1]["b"] = arrange_b(b_bwd)
    ident = make_ident()
    in_maps = []
    for c in range(N_CORES):
        d = c // 4
        seq0 = 16 * (c % 4)
        m = dict(wmaps[d])
        m["ident"] = ident
        m["x"] = arrange_x_core(x_rev if d else x, seq0)
        in_maps.append(m)

    res = bass_utils.run_bass_kernel_spmd(nc, in_maps,
                                          core_ids=list(range(N_CORES)))
    out = np.empty((N, T, 2 * H), np.float32)
    for c in range(N_CORES):
        d = c // 4
        seq0 = 16 * (c % 4)
        h = decode_out_core(res.results[c]["out0"], res.results[c]["out1"])
        if d == 0:
            out[seq0:seq0 + 16, :, :H] = h
        else:
            out[seq0:seq0 + 16, :, H:] = h[:, ::-1]
    return out
